# revision 10
# baseline (speedup 1.0000x reference)
"""Trainium2 Bass kernel for BiBo attention (GQA + per-head RMSNorm + RoPE +
SSMax scaling + causal attention + o_proj).

Sharding: tensor-parallel over the 4 KV-head groups x data-parallel over the
2 batch elements = 8 cores. Each core computes its 4 q-heads / 1 kv-head of
attention for one batch element plus its row-slice of o_proj; the host sums
the 4 partial o_proj outputs per batch element (row-parallel unshard).

Layout strategy (per core):
  - hidden^T [H, S] streamed from DRAM; projections produce q^T/k^T with the
    head dim on partitions so QK^T needs no transposes.
  - scores are computed transposed (scoresT[k, q]) so the PV matmul consumes
    exp(scoresT) directly; the softmax denominator is a ones-vector matmul
    (partition-dim sum on the PE) over quad-summed exp tiles; no
    max-subtraction is needed because RMS-normed q/k bound
    |scores| <= sqrt(HD)*ssmax*log(S) ~ 10.
  - causal structure: blocks fully below the diagonal are computed without
    any mask work; fully-masked blocks are skipped; the 4 diagonal blocks of
    each q-tile share one constant 128x128 triangular 0/1 bf16 mask applied
    to the exp tile on the vector engine, with QK/exp/PV narrowed to the
    live columns. Non-causal masks fall back to a generic additive path.
  - rstd = exp(-0.5*ln(var*sc + b)) on the scalar engine so the whole kernel
    uses a single activation table set (ln+exp); sqrt would thrash the
    table RAMs against exp.
"""

import math

import numpy as np

B, S, H = 2, 2048, 2048
NH, NKV, HD = 16, 4, 128
EPS = 1e-6
NCORES = 8
TP = 4            # kv-head groups
QH = NH // NKV    # q heads per core
SC = 512          # q-tile / s-chunk width
NSC = S // SC     # 4
KT = 128          # k tile
NKT = S // KT     # 16
HC = 128          # h contraction chunk
NHC = H // HC     # 16
SKIP_THRESH = -1e8

_compiled_cache = {}
LAST_EXEC_NS = None
LAST_RESULTS = None


def _enable_ldw_opt():
    import os
    if not os.environ.get("BASS_LDW_OPT"):
        return
    from concourse import bass_utils as bu
    if getattr(bu.run_command, "_ldw_patched", False):
        return
    orig = bu.run_command

    def patched(argv, **kw):
        argv = ["--enable-ldw-opt=true" if a == "--enable-ldw-opt=false" else a
                for a in argv]
        return orig(argv, **kw)

    patched._ldw_patched = True
    bu.run_command = patched


def _pin_act_table(arch, AF):
    """Restrict the activation-table chooser to the one set containing both
    ln and exp, so Ln/Exp/Square/Copy alternation never reloads tables.
    Mutates the functools-cached dict in place (emptied entries keep their
    index so act_func_set_id stays aligned with act_info.json)."""
    from concourse.hw_specs import get_activation_tables
    tabs = get_activation_tables(arch)
    keep = "natural_log_exp_and_others"
    needed = {AF.Exp, AF.Ln, AF.Square, AF.Copy}
    if keep in tabs and needed <= tabs[keep]:
        for name in list(tabs):
            if name != keep:
                tabs[name] = set()


def _build_program(plan, mask_counts):
    import concourse.mybir as mybir
    import concourse.tile as tile
    from concourse import bacc

    F32 = mybir.dt.float32
    MM = mybir.dt.bfloat16
    AF = mybir.ActivationFunctionType
    OP = mybir.AluOpType

    n_mask = sum(mask_counts)

    _enable_ldw_opt()
    nc = bacc.Bacc("TRN2", target_bir_lowering=False, debug=False,
                   num_devices=NCORES)
    _pin_act_table(nc.m.arch, AF)
    hT = nc.dram_tensor("hT", [NSC, 4, HC, 4 * SC], MM,
                        kind="ExternalInput").ap()
    wqT = nc.dram_tensor("wqT", [4, HC, 4 * QH * HD], MM,
                         kind="ExternalInput").ap()
    wkvT = nc.dram_tensor("wkvT", [2, HC, 8 * 2 * HD], MM,
                          kind="ExternalInput").ap()
    woT = nc.dram_tensor("woT", [QH * HD, H], MM, kind="ExternalInput").ap()
    cosT = nc.dram_tensor("cosT", [NSC, HD, SC], F32,
                          kind="ExternalInput").ap()
    sinT = nc.dram_tensor("sinT", [NSC, HD, SC], MM,
                          kind="ExternalInput").ap()
    qsc = nc.dram_tensor("qsc", [1, QH + 1], F32, kind="ExternalInput").ap()
    qsb = nc.dram_tensor("qsb", [1, QH + 1], F32, kind="ExternalInput").ap()
    iwq = nc.dram_tensor("iwq", [HD, 1], MM, kind="ExternalInput").ap()
    iwk = nc.dram_tensor("iwk", [HD, 1], MM, kind="ExternalInput").ap()
    tri01 = nc.dram_tensor("tri01", [KT, KT], MM, kind="ExternalInput").ap()
    if n_mask:
        mblk = nc.dram_tensor("mblk", [n_mask, KT, SC], F32,
                              kind="ExternalInput").ap()
    out = nc.dram_tensor("out", [S, H], MM, kind="ExternalOutput").ap()

    with tile.TileContext(nc) as tc:
        _emit(nc, tc, locals(), plan, mask_counts, MM, F32, AF, OP)
    nc.compile()
    return nc


def _emit(nc, tc, T, plan, mask_counts, MM, F32, AF, OP):
    from contextlib import ExitStack

    hT, wqT, wkvT, woT = T["hT"], T["wqT"], T["wkvT"], T["woT"]
    cosT, sinT = T["cosT"], T["sinT"]
    qsc, qsb = T["qsc"], T["qsb"]
    iwq, iwk, out = T["iwq"], T["iwk"], T["out"]
    tri01 = T["tri01"]
    mblk = T.get("mblk")

    ctx = ExitStack()
    with ctx:
        const = ctx.enter_context(tc.tile_pool(name="const", bufs=1))
        wpool = ctx.enter_context(tc.tile_pool(name="w", bufs=1))
        persist = ctx.enter_context(tc.tile_pool(name="persist", bufs=1))
        hpool = ctx.enter_context(tc.tile_pool(name="h", bufs=6))
        mpool = ctx.enter_context(tc.tile_pool(name="m", bufs=4))
        spool = ctx.enter_context(tc.tile_pool(name="s", bufs=2))
        epool = ctx.enter_context(tc.tile_pool(name="e", bufs=3))
        atpool = ctx.enter_context(tc.tile_pool(name="at", bufs=8))
        opool_sb = ctx.enter_context(tc.tile_pool(name="osb", bufs=2))
        ps_mm = ctx.enter_context(tc.tile_pool(name="psmm", bufs=3, space="PSUM"))
        ps_pv = ctx.enter_context(tc.tile_pool(name="pspv", bufs=2, space="PSUM"))
        ps_sm = ctx.enter_context(tc.tile_pool(name="pssm", bufs=1, space="PSUM"))
        ps_o = ctx.enter_context(tc.tile_pool(name="pso", bufs=2, space="PSUM"))

        # ---- persistent tiles (loads emitted by the driver below) -------
        wq_g = [wpool.tile([128, 4 * QH * HD], MM, name=f"wqg{g}",
                           tag=f"wqg{g}") for g in range(4)]
        wkv_g = [wpool.tile([128, 8 * 2 * HD], MM, name=f"wkvg{g}",
                            tag=f"wkvg{g}") for g in range(2)]
        wo_t = wpool.tile([128, QH * H], MM, tag="wo")
        wq_ts = [wq_g[c // 4][:, (c % 4) * QH * HD:(c % 4 + 1) * QH * HD]
                 for c in range(NHC)]
        wkv_ts = [wkv_g[c // 8][:, (c % 8) * 2 * HD:(c % 8 + 1) * 2 * HD]
                  for c in range(NHC)]
        cos_t = wpool.tile([128, NSC * SC], F32, tag="cos")
        sin_t = wpool.tile([128, NSC * SC], MM, tag="sin")
        cs_loaded = [False] * NSC

        def cos_sl(sc):
            return cos_t[:, sc * SC:(sc + 1) * SC]

        def sin_sl(sc):
            return sin_t[:, sc * SC:(sc + 1) * SC]

        def load_cs(sc):
            if not cs_loaded[sc]:
                nc.sync.dma_start(cos_t[:, sc * SC:(sc + 1) * SC], cosT[sc])
                nc.sync.dma_start(sin_t[:, sc * SC:(sc + 1) * SC], sinT[sc])
                cs_loaded[sc] = True

        qsc_t = const.tile([1, QH + 1], F32, tag="qsc")
        qsb_t = const.tile([1, QH + 1], F32, tag="qsb")
        iwq_t = const.tile([128, 1], MM, tag="iwq")
        iwk_t = const.tile([128, 1], MM, tag="iwk")
        ones_t = const.tile([128, 1], MM, tag="ones")
        tri_t = const.tile([128, KT], MM, tag="tri", name="tri")
        khat = persist.tile([128, S], MM, tag="khat")
        v_sb = persist.tile([128, S], MM, tag="v")
        qhat = [persist.tile([128, S], MM, name=f"qhat{i}", tag=f"qhat{i}")
                for i in range(QH)]

        def load_wkv():
            for g in range(2):
                nc.sync.dma_start(wkv_g[g][:], wkvT[g])

        def load_wq_consts():
            for g in range(4):
                nc.sync.dma_start(wq_g[g][:], wqT[g])
            nc.sync.dma_start(qsc_t[:], qsc[:])
            nc.sync.dma_start(qsb_t[:], qsb[:])
            nc.sync.dma_start(iwq_t[:], iwq[:])
            nc.sync.dma_start(iwk_t[:], iwk[:])
            nc.sync.dma_start(tri_t[:], tri01[:])
            nc.vector.memset(ones_t[:], 1.0)

        def load_wo():
            for f in range(QH):
                nc.sync.dma_start(wo_t[:, f * H:(f + 1) * H],
                                  woT[f * HD:(f + 1) * HD, :])

        # norm+rope staged: s1 (right after the proj matmuls) does the
        # rotate-copy + cos-product (the last PSUM reads, so the proj bank
        # frees early) and the square on the scalar engine; the var matmul
        # (s2) trails by one projection group; s3 finishes rstd + rope.
        def norm_s1(pp, sc):
            sh = spool.tile([128, SC], MM, tag="sh", name="sh")
            nc.vector.tensor_copy(sh[0:64, :], pp[64:128, :])
            nc.vector.tensor_copy(sh[64:128, :], pp[0:64, :])
            sq = spool.tile([128, SC], MM, tag="sq", name="sq")
            nc.scalar.activation(sq[:], pp[:], AF.Square)
            uu = spool.tile([128, SC], MM, tag="uu", name="uu")
            nc.vector.tensor_mul(uu[:], pp[:], cos_sl(sc))
            return sh, sq, uu

        def norm_s2(sq, iw_t):
            var = ps_sm.tile([1, SC], F32, tag="sm", name="var")
            nc.tensor.matmul(var[:], iw_t[:], sq[:], start=True, stop=True)
            return var

        def norm_s3(sh, uu, var, sc, hd, hat_dst):
            # rstd = (var*qsc + qsb)^-0.5 via ln+exp (single act table set)
            hi = 0 if hd is None else hd + 1
            lv = spool.tile([1, SC], F32, tag="lv", name="lv")
            nc.scalar.activation(lv[:], var[:], AF.Ln,
                                 bias=qsb_t[:, hi:hi + 1],
                                 scale=qsc_t[:, hi:hi + 1])
            rs = spool.tile([1, SC], MM, tag="rs", name="rs")
            nc.scalar.activation(rs[:], lv[:], AF.Exp, scale=-0.5)
            # tt = rot(x) * sin' (sign pre-folded into sin'), s = uu + tt
            tt = spool.tile([128, SC], MM, tag="tt", name="tt")
            nc.vector.tensor_mul(tt[:], sh[:], sin_sl(sc))
            bb = spool.tile([128, SC], MM, tag="bb", name="bb")
            nc.gpsimd.partition_broadcast(bb[:], rs[:], 128)
            nc.vector.tensor_add(tt[:], tt[:], uu[:])
            nc.vector.tensor_mul(hat_dst, tt[:], bb[:])

        # ---- projections, per s-chunk -----------------------------------
        def hts_load(sc):
            tiles = []
            for g in range(4):
                t = hpool.tile([128, 4 * SC], MM, tag="ht", name="ht")
                nc.sync.dma_start(t[:], hT[sc, g])
                tiles.append(t)
            return [tiles[c // 4][:, (c % 4) * SC:(c % 4 + 1) * SC]
                    for c in range(NHC)]

        def proj_chunk(sc, hts):
            # five projections (k, q0..q3), staged so each var matmul is
            # emitted after the NEXT projection's matmul group; v-proj is
            # emitted between k and q0 so it only depends on wkv + hT.
            specs = [(iwk_t, None, khat)] + [
                (iwq_t, hd, qhat[hd]) for hd in range(QH)]
            state = []  # (sh, sq, uu, spec)

            def do_mm(idx):
                pp = ps_mm.tile([128, SC], F32, tag="mm", name="pp")
                for c in range(NHC):
                    if idx == 0:
                        w_sl = wkv_ts[c][:, 0:HD]
                    else:
                        w_sl = wq_ts[c][:, (idx - 1) * HD:idx * HD]
                    nc.tensor.matmul(pp[:], w_sl, hts[c][:],
                                     start=(c == 0), stop=(c == NHC - 1))
                sh, sq, uu = norm_s1(pp, sc)
                state.append((sh, sq, uu, specs[idx]))

            def finish_one():
                sh, sq, uu, (iw_t, hd, dst) = state.pop(0)
                var = norm_s2(sq, iw_t)
                norm_s3(sh, uu, var, sc, hd, dst[:, sc * SC:(sc + 1) * SC])

            def v_proj():
                for ss in range(4):
                    vp = ps_o.tile([128, SC], F32, tag="o", name="vp")
                    for c in range(NHC):
                        nc.tensor.matmul(vp[:, 0:HD],
                                         hts[c][:, ss * 128:(ss + 1) * 128],
                                         wkv_ts[c][:, HD:2 * HD],
                                         start=(c == 0), stop=(c == NHC - 1))
                    col = (sc * 4 + ss) * 128
                    nc.vector.tensor_copy(v_sb[:, col:col + 128], vp[:, 0:HD])

            do_mm(0)
            v_proj()
            for idx in range(1, 5):
                do_mm(idx)
                finish_one()
            finish_one()

        # ---- attention + o_proj, per q-tile ------------------------------
        mask_starts = [sum(mask_counts[:i]) for i in range(NSC)]

        def attn_qtile(qi):
            ats = []
            mask_idx = mask_starts[qi]
            kts = [kt for kt in range(NKT) if plan[qi][kt] != "skip"]
            mtiles = {}
            for kt in kts:
                if plan[qi][kt] == "mask":
                    mt = mpool.tile([128, SC], F32, tag="mask", name="mk")
                    nc.sync.dma_start(mt[:], mblk[mask_idx])
                    mtiles[kt] = mt
                    mask_idx += 1
            for hd in range(QH):
                qsl = qhat[hd][:, qi * SC:(qi + 1) * SC]
                pv = ps_pv.tile([128, SC], F32, tag="pv")
                es = ps_sm.tile([1, SC], F32, tag="sm", name="es")
                sts = {}
                pend = []
                esn = [0, 0]  # groups emitted, total groups
                ngroups = (len(kts) + 3) // 4
                esn[1] = ngroups

                def tail(j):
                    kt = kts[j]
                    st, c0 = sts.pop(j)
                    ex = epool.tile([128, SC], MM, tag="ex", name="ex",
                                    bufs=5)
                    nc.scalar.activation(ex[:, c0:SC], st[:, c0:SC], AF.Exp)
                    if c0:
                        nc.vector.memset(ex[:, 0:c0], 0.0)
                        nc.vector.tensor_mul(ex[:, c0:c0 + KT],
                                             ex[:, c0:c0 + KT], tri_t[:])
                    elif plan[qi][kt] == "diag0":
                        nc.vector.tensor_mul(ex[:, 0:KT], ex[:, 0:KT],
                                             tri_t[:])
                    last = j == len(kts) - 1
                    nc.tensor.matmul(pv[:, c0:SC],
                                     v_sb[:, kt * 128:(kt + 1) * 128],
                                     ex[:, c0:SC], start=(j == 0), stop=last)
                    pend.append(ex)
                    if len(pend) == 4 or last:
                        if len(pend) == 1:
                            red = pend[0]
                        else:
                            red = epool.tile([128, SC], MM, tag="exs",
                                             name="exs", bufs=2)
                            nc.vector.tensor_add(red[:], pend[0][:],
                                                 pend[1][:])
                            if len(pend) == 4:
                                red2 = epool.tile([128, SC], MM, tag="exs2",
                                                  name="exs2", bufs=2)
                                nc.vector.tensor_add(red2[:], pend[2][:],
                                                     pend[3][:])
                                nc.vector.tensor_add(red[:], red[:], red2[:])
                            elif len(pend) == 3:
                                nc.vector.tensor_add(red[:], red[:],
                                                     pend[2][:])
                        nc.tensor.matmul(es[:], ones_t[:], red[:],
                                         start=(esn[0] == 0),
                                         stop=(esn[0] == esn[1] - 1))
                        esn[0] += 1
                        pend.clear()

                # pipeline QK^T one k-tile ahead of exp/PV
                for j, kt in enumerate(kts):
                    kind = plan[qi][kt]
                    c0 = int(kind[4]) * KT if kind.startswith("diag") else 0
                    st = ps_mm.tile([128, SC], F32, tag="mm")
                    nc.tensor.matmul(st[:, c0:SC],
                                     khat[:, kt * 128:(kt + 1) * 128],
                                     qsl[:, c0:SC], start=True, stop=True)
                    if kind == "mask":
                        nc.vector.tensor_add(st[:], st[:], mtiles[kt][:])
                    sts[j] = (st, c0)
                    if j >= 1:
                        tail(j - 1)
                tail(len(kts) - 1)
                rs = spool.tile([1, SC], F32, tag="ars")
                nc.vector.reciprocal_approx_fast(rs[:], es[:])
                bb = spool.tile([128, SC], F32, tag="abb")
                nc.gpsimd.partition_broadcast(bb[:], rs[:], 128)
                at = atpool.tile([128, SC], MM, tag="at")
                nc.vector.tensor_mul(at[:], pv[:], bb[:])
                ats.append(at)
            # o_proj for this q-tile
            for ss in range(4):
                ob = opool_sb.tile([128, H], MM, tag="osb", name="ob")
                for ho in range(4):
                    op_t = ps_o.tile([128, SC], F32, tag="o", name="op")
                    for hd in range(QH):
                        nc.tensor.matmul(
                            op_t[:],
                            ats[hd][:, ss * 128:(ss + 1) * 128],
                            wo_t[:, hd * H + ho * SC:hd * H + (ho + 1) * SC],
                            start=(hd == 0), stop=(hd == QH - 1))
                    if ho % 2 == 0:
                        nc.scalar.copy(ob[:, ho * SC:(ho + 1) * SC], op_t[:])
                    else:
                        nc.vector.tensor_copy(ob[:, ho * SC:(ho + 1) * SC],
                                              op_t[:])
                dma_q = nc.sync if qi == NSC - 1 else nc.gpsimd
                dma_q.dma_start(
                    out[qi * SC + ss * 128:qi * SC + (ss + 1) * 128, :],
                    ob[:])

        # ---- driver: software-pipelined phase order ----------------------
        load_wkv()
        hts0 = hts_load(0)
        load_cs(0)
        load_wq_consts()
        proj_chunk(0, hts0)
        hts1 = hts_load(1)
        load_cs(1)
        proj_chunk(1, hts1)
        load_wo()
        attn_qtile(0)
        hts2 = hts_load(2)
        load_cs(2)
        proj_chunk(2, hts2)
        attn_qtile(1)
        hts3 = hts_load(3)
        load_cs(3)
        proj_chunk(3, hts3)
        attn_qtile(2)
        attn_qtile(3)


def _causal_diag_j(blk, qi, kt):
    """Return j in 0..3 if the block matches the canonical causal step at
    diagonal offset (kt == 4*qi + j), else None. blk: [B, SC, KT]."""
    j = kt - 4 * qi
    if not (0 <= j <= 3):
        return None
    q_idx = qi * SC + np.arange(SC)[:, None]
    k_idx = kt * KT + np.arange(KT)[None, :]
    want = np.where(k_idx > q_idx, np.float32(-1e9), np.float32(0.0))
    return j if bool((blk == want[None]).all()) else None


def _mask_plan(mask):
    """Classify [qi][kt] blocks of the (q,k) mask, unified across batch."""
    plan = []
    for qi in range(NSC):
        row = []
        for kt in range(NKT):
            blk = mask[:, 0, qi * SC:(qi + 1) * SC, kt * KT:(kt + 1) * KT]
            if (blk <= SKIP_THRESH).all():
                row.append("skip")
            elif (blk == 0.0).all():
                row.append("zero")
            else:
                j = _causal_diag_j(blk, qi, kt)
                row.append(f"diag{j}" if j is not None else "mask")
        # guard: a q-tile with no included block would divide by zero
        if all(s == "skip" for s in row):
            row[0] = "mask"
        plan.append(row)
    return plan


def kernel(hidden_states, cos, sin, attention_mask, wq, wk, wv, wo,
           q_norm_w, k_norm_w, ssmax_scale):
    global LAST_EXEC_NS
    import os
    import ml_dtypes
    from concourse.bass_utils import run_bass_kernel_spmd

    f32 = np.float32
    hidden_states = np.asarray(hidden_states, f32)
    cos = np.asarray(cos, f32)
    sin = np.asarray(sin, f32)
    attention_mask = np.asarray(attention_mask, f32)
    wq = np.asarray(wq, f32)
    wk = np.asarray(wk, f32)
    wv = np.asarray(wv, f32)
    wo = np.asarray(wo, f32)
    q_norm_w = np.asarray(q_norm_w, f32)
    k_norm_w = np.asarray(k_norm_w, f32)
    ssmax = np.asarray(ssmax_scale, f32).reshape(NH)

    plan = _mask_plan(attention_mask)
    mask_counts = [sum(1 for s in row if s == "mask") for row in plan]
    key = (tuple(tuple(r) for r in plan),)
    if key not in _compiled_cache:
        _compiled_cache[key] = _build_program(plan, mask_counts)
    nc = _compiled_cache[key]

    bf16 = ml_dtypes.bfloat16
    qw = np.tile(q_norm_w, QH)
    iwq_np = (1.0 / (HD * q_norm_w ** 2)).astype(bf16)[:, None]
    iwk_np = (1.0 / (HD * k_norm_w ** 2)).astype(bf16)[:, None]
    # cos kept f32; sin gets rotate_half's sign fold: sin'[d<64] = -sin[d]
    sinp = sin.T.copy()                       # [HD, S]
    sinp[:64] = -sinp[:64]
    cos_np = np.ascontiguousarray(
        cos.T.reshape(HD, NSC, SC).transpose(1, 0, 2))        # [NSC, HD, SC]
    sin_np = np.ascontiguousarray(
        sinp.reshape(HD, NSC, SC).transpose(1, 0, 2)).astype(bf16)
    tri_np = (np.arange(KT)[:, None] <= np.arange(KT)[None, :]) \
        .astype(bf16)                                          # keep r <= c

    in_maps = []
    for core in range(NCORES):
        b, g = divmod(core, TP)
        hTm = np.ascontiguousarray(
            hidden_states[b].T.reshape(4, 4, HC, NSC, SC)
            .transpose(3, 0, 2, 1, 4).reshape(NSC, 4, HC, 4 * SC)
        ).astype(bf16)
        wq_s = wq[g * QH * HD:(g + 1) * QH * HD] * qw[:, None]
        wk_s = wk[g * HD:(g + 1) * HD] * k_norm_w[:, None]
        wv_s = wv[g * HD:(g + 1) * HD]
        wo_s = wo[:, g * QH * HD:(g + 1) * QH * HD]
        qcv = np.array([ssmax[g * QH + i] * math.log(S) / math.sqrt(HD)
                        for i in range(QH)], f32)
        # entry 0 is the k-norm (qc=1); entries 1..QH are the q heads
        qcall = np.concatenate([[1.0], qcv]).astype(f32)
        qsc_np = (1.0 / qcall ** 2)[None, :].astype(f32)
        qsb_np = (EPS / qcall ** 2)[None, :].astype(f32)
        wqTm = np.ascontiguousarray(
            wq_s.T.reshape(4, 4, HC, QH * HD)
            .transpose(0, 2, 1, 3).reshape(4, HC, 4 * QH * HD)).astype(bf16)
        wkv = np.concatenate(
            [wk_s.T.reshape(NHC, HC, 1, HD), wv_s.T.reshape(NHC, HC, 1, HD)],
            axis=2)  # [NHC, HC, 2, HD]
        wkvm = np.ascontiguousarray(
            wkv.reshape(2, 8, HC, 2 * HD).transpose(0, 2, 1, 3)
            .reshape(2, HC, 8 * 2 * HD)).astype(bf16)
        m = {
            "hT": hTm,
            "wqT": wqTm,
            "wkvT": wkvm,
            "woT": np.ascontiguousarray(wo_s.T).astype(bf16),
            "cosT": cos_np, "sinT": sin_np,
            "qsc": qsc_np, "qsb": qsb_np, "iwq": iwq_np, "iwk": iwk_np,
            "tri01": tri_np,
        }
        n_mask = sum(mask_counts)
        if n_mask:
            blocks = np.zeros((n_mask, KT, SC), f32)
            i = 0
            for qi in range(NSC):
                for kt in range(NKT):
                    if plan[qi][kt] != "mask":
                        continue
                    blocks[i] = attention_mask[
                        b, 0, qi * SC:(qi + 1) * SC,
                        kt * KT:(kt + 1) * KT].T
                    i += 1
            m["mblk"] = blocks
        in_maps.append(m)

    trace = bool(int(os.environ.get("BASS_KERNEL_TRACE", "0")))
    res = run_bass_kernel_spmd(nc, in_maps, list(range(NCORES)), trace=trace)
    LAST_EXEC_NS = res.exec_time_ns
    globals()["LAST_RESULTS"] = res

    final = np.zeros((B, S, H), f32)
    for core in range(NCORES):
        b = core // TP
        final[b] += np.asarray(res.results[core]["out"], f32)
    return final


# revision 13
# speedup vs baseline: 1.1773x; 1.1773x over previous
"""Trainium2 Bass kernel for BiBo attention (GQA + per-head RMSNorm + RoPE +
SSMax scaling + causal attention + o_proj).

Sharding: tensor-parallel over the 4 KV-head groups x data-parallel over the
2 batch elements = 8 cores. Each core computes its 4 q-heads / 1 kv-head of
attention for one batch element plus its row-slice of o_proj; the host sums
the 4 partial o_proj outputs per batch element (row-parallel unshard).

Layout strategy (per core):
  - hidden^T [H, S] streamed from DRAM; projections produce q^T/k^T with the
    head dim on partitions so QK^T needs no transposes.
  - scores are computed transposed (scoresT[k, q]) so the PV matmul consumes
    exp(scoresT) directly; the softmax denominator is a ones-vector matmul
    (partition-dim sum on the PE) over quad-summed exp tiles; no
    max-subtraction is needed because RMS-normed q/k bound
    |scores| <= sqrt(HD)*ssmax*log(S) ~ 10.
  - causal structure: blocks fully below the diagonal are computed without
    any mask work; fully-masked blocks are skipped; the 4 diagonal blocks of
    each q-tile share one constant 128x128 triangular 0/1 bf16 mask applied
    to the exp tile on the vector engine, with QK/exp/PV narrowed to the
    live columns. Non-causal masks fall back to a generic additive path.
  - rstd = exp(-0.5*ln(var*sc + b)) on the scalar engine so the whole kernel
    uses a single activation table set (ln+exp); sqrt would thrash the
    table RAMs against exp.
"""

import math

import numpy as np

B, S, H = 2, 2048, 2048
NH, NKV, HD = 16, 4, 128
EPS = 1e-6
NCORES = 8
TP = 4            # kv-head groups
QH = NH // NKV    # q heads per core
SC = 512          # q-tile / s-chunk width
NSC = S // SC     # 4
KT = 128          # k tile
NKT = S // KT     # 16
HC = 128          # h contraction chunk
NHC = H // HC     # 16
SKIP_THRESH = -1e8

_compiled_cache = {}
LAST_EXEC_NS = None
LAST_RESULTS = None


def _enable_ldw_opt():
    import os
    if not os.environ.get("BASS_LDW_OPT"):
        return
    from concourse import bass_utils as bu
    if getattr(bu.run_command, "_ldw_patched", False):
        return
    orig = bu.run_command

    def patched(argv, **kw):
        argv = ["--enable-ldw-opt=true" if a == "--enable-ldw-opt=false" else a
                for a in argv]
        return orig(argv, **kw)

    patched._ldw_patched = True
    bu.run_command = patched


def _pin_act_table(arch, AF):
    """Restrict the activation-table chooser to the one set containing both
    ln and exp, so Ln/Exp/Square/Copy alternation never reloads tables.
    Mutates the functools-cached dict in place (emptied entries keep their
    index so act_func_set_id stays aligned with act_info.json)."""
    from concourse.hw_specs import get_activation_tables
    tabs = get_activation_tables(arch)
    keep = "natural_log_exp_and_others"
    needed = {AF.Exp, AF.Ln, AF.Square, AF.Copy}
    if keep in tabs and needed <= tabs[keep]:
        for name in list(tabs):
            if name != keep:
                tabs[name] = set()


def _build_program(plan, mask_counts):
    import concourse.mybir as mybir
    import concourse.tile as tile
    from concourse import bacc

    F32 = mybir.dt.float32
    MM = mybir.dt.bfloat16
    AF = mybir.ActivationFunctionType
    OP = mybir.AluOpType

    n_mask = sum(mask_counts)

    _enable_ldw_opt()
    nc = bacc.Bacc("TRN2", target_bir_lowering=False, debug=False,
                   num_devices=NCORES)
    _pin_act_table(nc.m.arch, AF)
    hT = nc.dram_tensor("hT", [NSC, 4, HC, 4 * SC], MM,
                        kind="ExternalInput").ap()
    wqT = nc.dram_tensor("wqT", [4, HC, 4 * QH * HD], MM,
                         kind="ExternalInput").ap()
    wkvT = nc.dram_tensor("wkvT", [2, HC, 8 * 2 * HD], MM,
                          kind="ExternalInput").ap()
    woT = nc.dram_tensor("woT", [QH * HD, H], MM, kind="ExternalInput").ap()
    cosT = nc.dram_tensor("cosT", [NSC, HD, SC], F32,
                          kind="ExternalInput").ap()
    sinT = nc.dram_tensor("sinT", [NSC, HD, SC], MM,
                          kind="ExternalInput").ap()
    qsc = nc.dram_tensor("qsc", [1, QH + 1], F32, kind="ExternalInput").ap()
    qsb = nc.dram_tensor("qsb", [1, QH + 1], F32, kind="ExternalInput").ap()
    iwq = nc.dram_tensor("iwq", [HD, 1], MM, kind="ExternalInput").ap()
    iwk = nc.dram_tensor("iwk", [HD, 1], MM, kind="ExternalInput").ap()
    tri01 = nc.dram_tensor("tri01", [KT, KT], MM, kind="ExternalInput").ap()
    if n_mask:
        mblk = nc.dram_tensor("mblk", [n_mask, KT, SC], F32,
                              kind="ExternalInput").ap()
    out = nc.dram_tensor("out", [S, H], MM, kind="ExternalOutput").ap()

    with tile.TileContext(nc) as tc:
        _emit(nc, tc, locals(), plan, mask_counts, MM, F32, AF, OP)
    nc.compile()
    return nc


def _emit(nc, tc, T, plan, mask_counts, MM, F32, AF, OP):
    from contextlib import ExitStack

    hT, wqT, wkvT, woT = T["hT"], T["wqT"], T["wkvT"], T["woT"]
    cosT, sinT = T["cosT"], T["sinT"]
    qsc, qsb = T["qsc"], T["qsb"]
    iwq, iwk, out = T["iwq"], T["iwk"], T["out"]
    tri01 = T["tri01"]
    mblk = T.get("mblk")

    ctx = ExitStack()
    with ctx:
        const = ctx.enter_context(tc.tile_pool(name="const", bufs=1))
        wpool = ctx.enter_context(tc.tile_pool(name="w", bufs=1))
        persist = ctx.enter_context(tc.tile_pool(name="persist", bufs=1))
        hpool = ctx.enter_context(tc.tile_pool(name="h", bufs=6))
        mpool = ctx.enter_context(tc.tile_pool(name="m", bufs=4))
        spool = ctx.enter_context(tc.tile_pool(name="s", bufs=2))
        epool = ctx.enter_context(tc.tile_pool(name="e", bufs=3))
        atpool = ctx.enter_context(tc.tile_pool(name="at", bufs=8))
        opool_sb = ctx.enter_context(tc.tile_pool(name="osb", bufs=2))
        ps_mm = ctx.enter_context(tc.tile_pool(name="psmm", bufs=3, space="PSUM"))
        ps_pv = ctx.enter_context(tc.tile_pool(name="pspv", bufs=2, space="PSUM"))
        ps_sm = ctx.enter_context(tc.tile_pool(name="pssm", bufs=1, space="PSUM"))
        ps_o = ctx.enter_context(tc.tile_pool(name="pso", bufs=2, space="PSUM"))

        # ---- persistent tiles (loads emitted by the driver below) -------
        wq_g = [wpool.tile([128, 4 * QH * HD], MM, name=f"wqg{g}",
                           tag=f"wqg{g}") for g in range(4)]
        wkv_g = [wpool.tile([128, 8 * 2 * HD], MM, name=f"wkvg{g}",
                            tag=f"wkvg{g}") for g in range(2)]
        wo_t = wpool.tile([128, QH * H], MM, tag="wo")
        wq_ts = [wq_g[c // 4][:, (c % 4) * QH * HD:(c % 4 + 1) * QH * HD]
                 for c in range(NHC)]
        wkv_ts = [wkv_g[c // 8][:, (c % 8) * 2 * HD:(c % 8 + 1) * 2 * HD]
                  for c in range(NHC)]
        cos_t = wpool.tile([128, NSC * SC], F32, tag="cos")
        sin_t = wpool.tile([128, NSC * SC], MM, tag="sin")
        cs_loaded = [False] * NSC

        def cos_sl(sc):
            return cos_t[:, sc * SC:(sc + 1) * SC]

        def sin_sl(sc):
            return sin_t[:, sc * SC:(sc + 1) * SC]

        def load_cs(sc):
            if not cs_loaded[sc]:
                nc.sync.dma_start(cos_t[:, sc * SC:(sc + 1) * SC], cosT[sc])
                nc.sync.dma_start(sin_t[:, sc * SC:(sc + 1) * SC], sinT[sc])
                cs_loaded[sc] = True

        qsc_t = const.tile([1, QH + 1], F32, tag="qsc")
        qsb_t = const.tile([1, QH + 1], F32, tag="qsb")
        iwq_t = const.tile([128, 1], MM, tag="iwq")
        iwk_t = const.tile([128, 1], MM, tag="iwk")
        ones_t = const.tile([128, 1], MM, tag="ones")
        tri_t = const.tile([128, KT], MM, tag="tri", name="tri")
        khat = persist.tile([128, S], MM, tag="khat")
        v_sb = persist.tile([128, S], MM, tag="v")
        qhat = [persist.tile([128, S], MM, name=f"qhat{i}", tag=f"qhat{i}")
                for i in range(QH)]

        def load_wkv():
            for g in range(2):
                nc.sync.dma_start(wkv_g[g][:], wkvT[g])

        def load_wq_consts():
            for g in range(4):
                nc.sync.dma_start(wq_g[g][:], wqT[g])
            nc.sync.dma_start(qsc_t[:], qsc[:])
            nc.sync.dma_start(qsb_t[:], qsb[:])
            nc.sync.dma_start(iwq_t[:], iwq[:])
            nc.sync.dma_start(iwk_t[:], iwk[:])
            nc.sync.dma_start(tri_t[:], tri01[:])
            nc.vector.memset(ones_t[:], 1.0)

        def load_wo():
            for f in range(QH):
                nc.sync.dma_start(wo_t[:, f * H:(f + 1) * H],
                                  woT[f * HD:(f + 1) * HD, :])

        # norm+rope staged: s1 (right after the proj matmuls) does the
        # rotate-copy + cos-product (the last PSUM reads, so the proj bank
        # frees early) and the square on the scalar engine; the var matmul
        # (s2) trails by one projection group; s3 finishes rstd + rope.
        def norm_s1(pp, sc):
            sh = spool.tile([128, SC], MM, tag="sh", name="sh")
            nc.vector.tensor_copy(sh[0:64, :], pp[64:128, :])
            nc.vector.tensor_copy(sh[64:128, :], pp[0:64, :])
            sq = spool.tile([128, SC], MM, tag="sq", name="sq")
            nc.scalar.activation(sq[:], pp[:], AF.Square)
            uu = spool.tile([128, SC], MM, tag="uu", name="uu")
            nc.vector.tensor_mul(uu[:], pp[:], cos_sl(sc))
            return sh, sq, uu

        def norm_s2(sq, iw_t):
            var = ps_mm.tile([1, SC], F32, tag="mm", name="var")
            nc.tensor.matmul(var[:], iw_t[:], sq[:], start=True, stop=True)
            return var

        def norm_s3(sh, uu, var, sc, hd, hat_dst):
            # rstd = (var*qsc + qsb)^-0.5 via ln+exp (single act table set)
            hi = 0 if hd is None else hd + 1
            lv = spool.tile([1, SC], F32, tag="lv", name="lv")
            nc.scalar.activation(lv[:], var[:], AF.Ln,
                                 bias=qsb_t[:, hi:hi + 1],
                                 scale=qsc_t[:, hi:hi + 1])
            rs = spool.tile([1, SC], MM, tag="rs", name="rs")
            nc.scalar.activation(rs[:], lv[:], AF.Exp, scale=-0.5)
            # tt = rot(x) * sin' (sign pre-folded into sin'), s = uu + tt
            tt = spool.tile([128, SC], MM, tag="tt", name="tt")
            nc.vector.tensor_mul(tt[:], sh[:], sin_sl(sc))
            bb = spool.tile([128, SC], MM, tag="bb", name="bb")
            nc.gpsimd.partition_broadcast(bb[:], rs[:], 128)
            nc.vector.tensor_add(tt[:], tt[:], uu[:])
            nc.vector.tensor_mul(hat_dst, tt[:], bb[:])

        # ---- projections, per s-chunk -----------------------------------
        def hts_load(sc):
            tiles = []
            for g in range(4):
                t = hpool.tile([128, 4 * SC], MM, tag="ht", name="ht")
                nc.sync.dma_start(t[:], hT[sc, g])
                tiles.append(t)
            return [tiles[c // 4][:, (c % 4) * SC:(c % 4 + 1) * SC]
                    for c in range(NHC)]

        def proj_chunk(sc, hts):
            # five projections (k, q0..q3), staged so each var matmul is
            # emitted after the NEXT projection's matmul group; v-proj is
            # emitted between k and q0 so it only depends on wkv + hT.
            specs = [(iwk_t, None, khat)] + [
                (iwq_t, hd, qhat[hd]) for hd in range(QH)]
            state = []  # (sh, sq, uu, spec)

            def do_mm(idx):
                pp = ps_mm.tile([128, SC], F32, tag="mm", name="pp")
                for c in range(NHC):
                    if idx == 0:
                        w_sl = wkv_ts[c][:, 0:HD]
                    else:
                        w_sl = wq_ts[c][:, (idx - 1) * HD:idx * HD]
                    nc.tensor.matmul(pp[:], w_sl, hts[c][:],
                                     start=(c == 0), stop=(c == NHC - 1))
                sh, sq, uu = norm_s1(pp, sc)
                state.append((sh, sq, uu, specs[idx]))

            def finish_one():
                sh, sq, uu, (iw_t, hd, dst) = state.pop(0)
                var = norm_s2(sq, iw_t)
                norm_s3(sh, uu, var, sc, hd, dst[:, sc * SC:(sc + 1) * SC])

            def v_proj():
                for ss in range(4):
                    vp = ps_o.tile([128, SC], F32, tag="o", name="vp")
                    for c in range(NHC):
                        nc.tensor.matmul(vp[:, 0:HD],
                                         hts[c][:, ss * 128:(ss + 1) * 128],
                                         wkv_ts[c][:, HD:2 * HD],
                                         start=(c == 0), stop=(c == NHC - 1))
                    col = (sc * 4 + ss) * 128
                    nc.vector.tensor_copy(v_sb[:, col:col + 128], vp[:, 0:HD])

            do_mm(0)
            v_proj()
            for idx in range(1, 5):
                do_mm(idx)
                finish_one()
            finish_one()

        # ---- attention + o_proj, per q-tile ------------------------------
        mask_starts = [sum(mask_counts[:i]) for i in range(NSC)]

        def attn_qtile(qi):
            ats = []
            mask_idx = mask_starts[qi]
            kts = [kt for kt in range(NKT) if plan[qi][kt] != "skip"]
            mtiles = {}
            for kt in kts:
                if plan[qi][kt] == "mask":
                    mt = mpool.tile([128, SC], F32, tag="mask", name="mk")
                    nc.sync.dma_start(mt[:], mblk[mask_idx])
                    mtiles[kt] = mt
                    mask_idx += 1
            for hd in range(QH):
                qsl = qhat[hd][:, qi * SC:(qi + 1) * SC]
                pv = ps_pv.tile([128, SC], F32, tag="pv")
                es = ps_sm.tile([1, SC], F32, tag="es")
                sts = {}
                pend = []
                esn = [0, 0]  # groups emitted, total groups
                ngroups = (len(kts) + 3) // 4
                esn[1] = ngroups

                def tail(j):
                    kt = kts[j]
                    st, c0 = sts.pop(j)
                    ex = epool.tile([128, SC], MM, tag="ex", name="ex",
                                    bufs=5)
                    nc.scalar.activation(ex[:, c0:SC], st[:, c0:SC], AF.Exp)
                    if c0:
                        nc.vector.memset(ex[:, 0:c0], 0.0)
                        nc.vector.tensor_mul(ex[:, c0:c0 + KT],
                                             ex[:, c0:c0 + KT], tri_t[:])
                    elif plan[qi][kt] == "diag0":
                        nc.vector.tensor_mul(ex[:, 0:KT], ex[:, 0:KT],
                                             tri_t[:])
                    last = j == len(kts) - 1
                    nc.tensor.matmul(pv[:, c0:SC],
                                     v_sb[:, kt * 128:(kt + 1) * 128],
                                     ex[:, c0:SC], start=(j == 0), stop=last)
                    pend.append(ex)
                    if len(pend) == 4 or last:
                        if len(pend) == 1:
                            red = pend[0]
                        else:
                            red = epool.tile([128, SC], MM, tag="exs",
                                             name="exs", bufs=2)
                            nc.vector.tensor_add(red[:], pend[0][:],
                                                 pend[1][:])
                            if len(pend) == 4:
                                red2 = epool.tile([128, SC], MM, tag="exs2",
                                                  name="exs2", bufs=2)
                                nc.vector.tensor_add(red2[:], pend[2][:],
                                                     pend[3][:])
                                nc.vector.tensor_add(red[:], red[:], red2[:])
                            elif len(pend) == 3:
                                nc.vector.tensor_add(red[:], red[:],
                                                     pend[2][:])
                        nc.tensor.matmul(es[:], ones_t[:], red[:],
                                         start=(esn[0] == 0),
                                         stop=(esn[0] == esn[1] - 1))
                        esn[0] += 1
                        pend.clear()

                # pipeline QK^T one k-tile ahead of exp/PV
                for j, kt in enumerate(kts):
                    kind = plan[qi][kt]
                    c0 = int(kind[4]) * KT if kind.startswith("diag") else 0
                    st = ps_mm.tile([128, SC], F32, tag="mm")
                    nc.tensor.matmul(st[:, c0:SC],
                                     khat[:, kt * 128:(kt + 1) * 128],
                                     qsl[:, c0:SC], start=True, stop=True)
                    if kind == "mask":
                        nc.vector.tensor_add(st[:], st[:], mtiles[kt][:])
                    sts[j] = (st, c0)
                    if j >= 1:
                        tail(j - 1)
                tail(len(kts) - 1)
                rs = spool.tile([1, SC], F32, tag="ars")
                nc.vector.reciprocal_approx_fast(rs[:], es[:])
                bb = spool.tile([128, SC], F32, tag="abb")
                nc.gpsimd.partition_broadcast(bb[:], rs[:], 128)
                at = atpool.tile([128, SC], MM, tag="at")
                nc.vector.tensor_mul(at[:], pv[:], bb[:])
                ats.append(at)
            # o_proj for this q-tile
            for ss in range(4):
                ob = opool_sb.tile([128, H], MM, tag="osb", name="ob")
                for ho in range(4):
                    op_t = ps_o.tile([128, SC], F32, tag="o", name="op")
                    for hd in range(QH):
                        nc.tensor.matmul(
                            op_t[:],
                            ats[hd][:, ss * 128:(ss + 1) * 128],
                            wo_t[:, hd * H + ho * SC:hd * H + (ho + 1) * SC],
                            start=(hd == 0), stop=(hd == QH - 1))
                    if ho % 2 == 0:
                        nc.scalar.copy(ob[:, ho * SC:(ho + 1) * SC], op_t[:])
                    else:
                        nc.vector.tensor_copy(ob[:, ho * SC:(ho + 1) * SC],
                                              op_t[:])
                nc.gpsimd.dma_start(
                    out[qi * SC + ss * 128:qi * SC + (ss + 1) * 128, :],
                    ob[:])

        # ---- driver: software-pipelined phase order ----------------------
        load_wkv()
        hts0 = hts_load(0)
        load_cs(0)
        load_wq_consts()
        proj_chunk(0, hts0)
        hts1 = hts_load(1)
        load_cs(1)
        proj_chunk(1, hts1)
        load_wo()
        attn_qtile(0)
        hts2 = hts_load(2)
        load_cs(2)
        proj_chunk(2, hts2)
        attn_qtile(1)
        hts3 = hts_load(3)
        load_cs(3)
        proj_chunk(3, hts3)
        attn_qtile(2)
        attn_qtile(3)


def _causal_diag_j(blk, qi, kt):
    """Return j in 0..3 if the block matches the canonical causal step at
    diagonal offset (kt == 4*qi + j), else None. blk: [B, SC, KT]."""
    j = kt - 4 * qi
    if not (0 <= j <= 3):
        return None
    q_idx = qi * SC + np.arange(SC)[:, None]
    k_idx = kt * KT + np.arange(KT)[None, :]
    want = np.where(k_idx > q_idx, np.float32(-1e9), np.float32(0.0))
    return j if bool((blk == want[None]).all()) else None


def _mask_plan(mask):
    """Classify [qi][kt] blocks of the (q,k) mask, unified across batch."""
    plan = []
    for qi in range(NSC):
        row = []
        for kt in range(NKT):
            blk = mask[:, 0, qi * SC:(qi + 1) * SC, kt * KT:(kt + 1) * KT]
            if (blk <= SKIP_THRESH).all():
                row.append("skip")
            elif (blk == 0.0).all():
                row.append("zero")
            else:
                j = _causal_diag_j(blk, qi, kt)
                row.append(f"diag{j}" if j is not None else "mask")
        # guard: a q-tile with no included block would divide by zero
        if all(s == "skip" for s in row):
            row[0] = "mask"
        plan.append(row)
    return plan


def kernel(hidden_states, cos, sin, attention_mask, wq, wk, wv, wo,
           q_norm_w, k_norm_w, ssmax_scale):
    global LAST_EXEC_NS
    import os
    import ml_dtypes
    from concourse.bass_utils import run_bass_kernel_spmd

    f32 = np.float32
    hidden_states = np.asarray(hidden_states, f32)
    cos = np.asarray(cos, f32)
    sin = np.asarray(sin, f32)
    attention_mask = np.asarray(attention_mask, f32)
    wq = np.asarray(wq, f32)
    wk = np.asarray(wk, f32)
    wv = np.asarray(wv, f32)
    wo = np.asarray(wo, f32)
    q_norm_w = np.asarray(q_norm_w, f32)
    k_norm_w = np.asarray(k_norm_w, f32)
    ssmax = np.asarray(ssmax_scale, f32).reshape(NH)

    plan = _mask_plan(attention_mask)
    mask_counts = [sum(1 for s in row if s == "mask") for row in plan]
    key = (tuple(tuple(r) for r in plan),)
    if key not in _compiled_cache:
        _compiled_cache[key] = _build_program(plan, mask_counts)
    nc = _compiled_cache[key]

    bf16 = ml_dtypes.bfloat16
    qw = np.tile(q_norm_w, QH)
    iwq_np = (1.0 / (HD * q_norm_w ** 2)).astype(bf16)[:, None]
    iwk_np = (1.0 / (HD * k_norm_w ** 2)).astype(bf16)[:, None]
    # cos kept f32; sin gets rotate_half's sign fold: sin'[d<64] = -sin[d]
    sinp = sin.T.copy()                       # [HD, S]
    sinp[:64] = -sinp[:64]
    cos_np = np.ascontiguousarray(
        cos.T.reshape(HD, NSC, SC).transpose(1, 0, 2))        # [NSC, HD, SC]
    sin_np = np.ascontiguousarray(
        sinp.reshape(HD, NSC, SC).transpose(1, 0, 2)).astype(bf16)
    tri_np = (np.arange(KT)[:, None] <= np.arange(KT)[None, :]) \
        .astype(bf16)                                          # keep r <= c

    in_maps = []
    for core in range(NCORES):
        b, g = divmod(core, TP)
        hTm = np.ascontiguousarray(
            hidden_states[b].T.reshape(4, 4, HC, NSC, SC)
            .transpose(3, 0, 2, 1, 4).reshape(NSC, 4, HC, 4 * SC)
        ).astype(bf16)
        wq_s = wq[g * QH * HD:(g + 1) * QH * HD] * qw[:, None]
        wk_s = wk[g * HD:(g + 1) * HD] * k_norm_w[:, None]
        wv_s = wv[g * HD:(g + 1) * HD]
        wo_s = wo[:, g * QH * HD:(g + 1) * QH * HD]
        qcv = np.array([ssmax[g * QH + i] * math.log(S) / math.sqrt(HD)
                        for i in range(QH)], f32)
        # entry 0 is the k-norm (qc=1); entries 1..QH are the q heads
        qcall = np.concatenate([[1.0], qcv]).astype(f32)
        qsc_np = (1.0 / qcall ** 2)[None, :].astype(f32)
        qsb_np = (EPS / qcall ** 2)[None, :].astype(f32)
        wqTm = np.ascontiguousarray(
            wq_s.T.reshape(4, 4, HC, QH * HD)
            .transpose(0, 2, 1, 3).reshape(4, HC, 4 * QH * HD)).astype(bf16)
        wkv = np.concatenate(
            [wk_s.T.reshape(NHC, HC, 1, HD), wv_s.T.reshape(NHC, HC, 1, HD)],
            axis=2)  # [NHC, HC, 2, HD]
        wkvm = np.ascontiguousarray(
            wkv.reshape(2, 8, HC, 2 * HD).transpose(0, 2, 1, 3)
            .reshape(2, HC, 8 * 2 * HD)).astype(bf16)
        m = {
            "hT": hTm,
            "wqT": wqTm,
            "wkvT": wkvm,
            "woT": np.ascontiguousarray(wo_s.T).astype(bf16),
            "cosT": cos_np, "sinT": sin_np,
            "qsc": qsc_np, "qsb": qsb_np, "iwq": iwq_np, "iwk": iwk_np,
            "tri01": tri_np,
        }
        n_mask = sum(mask_counts)
        if n_mask:
            blocks = np.zeros((n_mask, KT, SC), f32)
            i = 0
            for qi in range(NSC):
                for kt in range(NKT):
                    if plan[qi][kt] != "mask":
                        continue
                    blocks[i] = attention_mask[
                        b, 0, qi * SC:(qi + 1) * SC,
                        kt * KT:(kt + 1) * KT].T
                    i += 1
            m["mblk"] = blocks
        in_maps.append(m)

    trace = bool(int(os.environ.get("BASS_KERNEL_TRACE", "0")))
    res = run_bass_kernel_spmd(nc, in_maps, list(range(NCORES)), trace=trace)
    LAST_EXEC_NS = res.exec_time_ns
    globals()["LAST_RESULTS"] = res

    final = np.zeros((B, S, H), f32)
    for core in range(NCORES):
        b = core // TP
        final[b] += np.asarray(res.results[core]["out"], f32)
    return final


# revision 20
# speedup vs baseline: 1.1892x; 1.0101x over previous
"""Trainium2 Bass kernel for BiBo attention (GQA + per-head RMSNorm + RoPE +
SSMax scaling + causal attention + o_proj).

Sharding: tensor-parallel over the 4 KV-head groups x data-parallel over the
2 batch elements = 8 cores. Each core computes its 4 q-heads / 1 kv-head of
attention for one batch element plus its row-slice of o_proj; the host sums
the 4 partial o_proj outputs per batch element (row-parallel unshard).

Layout strategy (per core):
  - hidden^T [H, S] streamed from DRAM; projections produce q^T/k^T with the
    head dim on partitions so QK^T needs no transposes.
  - scores are computed transposed (scoresT[k, q]) so the PV matmul consumes
    exp(scoresT) directly; the softmax denominator is a ones-vector matmul
    (partition-dim sum on the PE) over quad-summed exp tiles; no
    max-subtraction is needed because RMS-normed q/k bound
    |scores| <= sqrt(HD)*ssmax*log(S) ~ 10.
  - causal structure: blocks fully below the diagonal are computed without
    any mask work; fully-masked blocks are skipped; the 4 diagonal blocks of
    each q-tile share one constant 128x128 triangular 0/1 bf16 mask applied
    to the exp tile on the vector engine, with QK/exp/PV narrowed to the
    live columns. Non-causal masks fall back to a generic additive path.
  - rstd = exp(-0.5*ln(var*sc + b)) on the scalar engine so the whole kernel
    uses a single activation table set (ln+exp); sqrt would thrash the
    table RAMs against exp.
"""

import math

import numpy as np

B, S, H = 2, 2048, 2048
NH, NKV, HD = 16, 4, 128
EPS = 1e-6
NCORES = 8
TP = 4            # kv-head groups
QH = NH // NKV    # q heads per core
SC = 512          # q-tile / s-chunk width
NSC = S // SC     # 4
KT = 128          # k tile
NKT = S // KT     # 16
HC = 128          # h contraction chunk
NHC = H // HC     # 16
SKIP_THRESH = -1e8

_compiled_cache = {}
LAST_EXEC_NS = None
LAST_RESULTS = None


def _enable_ldw_opt():
    import os
    if not os.environ.get("BASS_LDW_OPT"):
        return
    from concourse import bass_utils as bu
    if getattr(bu.run_command, "_ldw_patched", False):
        return
    orig = bu.run_command

    def patched(argv, **kw):
        argv = ["--enable-ldw-opt=true" if a == "--enable-ldw-opt=false" else a
                for a in argv]
        return orig(argv, **kw)

    patched._ldw_patched = True
    bu.run_command = patched


def _pin_act_table(arch, AF):
    """Restrict the activation-table chooser to the one set containing both
    ln and exp, so Ln/Exp/Square/Copy alternation never reloads tables.
    Mutates the functools-cached dict in place (emptied entries keep their
    index so act_func_set_id stays aligned with act_info.json)."""
    from concourse.hw_specs import get_activation_tables
    tabs = get_activation_tables(arch)
    keep = "natural_log_exp_and_others"
    needed = {AF.Exp, AF.Ln, AF.Square, AF.Copy}
    if keep in tabs and needed <= tabs[keep]:
        for name in list(tabs):
            if name != keep:
                tabs[name] = set()


def _build_program(plan, mask_counts):
    import concourse.mybir as mybir
    import concourse.tile as tile
    from concourse import bacc

    F32 = mybir.dt.float32
    MM = mybir.dt.bfloat16
    AF = mybir.ActivationFunctionType
    OP = mybir.AluOpType

    n_mask = sum(mask_counts)

    _enable_ldw_opt()
    nc = bacc.Bacc("TRN2", target_bir_lowering=False, debug=False,
                   num_devices=NCORES)
    _pin_act_table(nc.m.arch, AF)
    hT = nc.dram_tensor("hT", [NSC, 4, HC, 4 * SC], MM,
                        kind="ExternalInput").ap()
    wqT = nc.dram_tensor("wqT", [4, HC, 4 * QH * HD], MM,
                         kind="ExternalInput").ap()
    wkvT = nc.dram_tensor("wkvT", [2, HC, 8 * 2 * HD], MM,
                          kind="ExternalInput").ap()
    woT = nc.dram_tensor("woT", [QH * HD, H], MM, kind="ExternalInput").ap()
    cosT = nc.dram_tensor("cosT", [NSC, HD, SC], F32,
                          kind="ExternalInput").ap()
    sinT = nc.dram_tensor("sinT", [NSC, HD, SC], MM,
                          kind="ExternalInput").ap()
    qsc = nc.dram_tensor("qsc", [1, QH + 1], F32, kind="ExternalInput").ap()
    qsb = nc.dram_tensor("qsb", [1, QH + 1], F32, kind="ExternalInput").ap()
    iwq = nc.dram_tensor("iwq", [HD, 1], MM, kind="ExternalInput").ap()
    iwk = nc.dram_tensor("iwk", [HD, 1], MM, kind="ExternalInput").ap()
    tri01 = nc.dram_tensor("tri01", [KT, KT], MM, kind="ExternalInput").ap()
    if n_mask:
        mblk = nc.dram_tensor("mblk", [n_mask, KT, SC], F32,
                              kind="ExternalInput").ap()
    out = nc.dram_tensor("out", [S, H], MM, kind="ExternalOutput").ap()

    with tile.TileContext(nc) as tc:
        _emit(nc, tc, locals(), plan, mask_counts, MM, F32, AF, OP)
    nc.compile()
    return nc


def _emit(nc, tc, T, plan, mask_counts, MM, F32, AF, OP):
    from contextlib import ExitStack

    hT, wqT, wkvT, woT = T["hT"], T["wqT"], T["wkvT"], T["woT"]
    cosT, sinT = T["cosT"], T["sinT"]
    qsc, qsb = T["qsc"], T["qsb"]
    iwq, iwk, out = T["iwq"], T["iwk"], T["out"]
    tri01 = T["tri01"]
    mblk = T.get("mblk")

    ctx = ExitStack()
    with ctx:
        const = ctx.enter_context(tc.tile_pool(name="const", bufs=1))
        wpool = ctx.enter_context(tc.tile_pool(name="w", bufs=1))
        persist = ctx.enter_context(tc.tile_pool(name="persist", bufs=1))
        hpool = ctx.enter_context(tc.tile_pool(name="h", bufs=6))
        mpool = ctx.enter_context(tc.tile_pool(name="m", bufs=4))
        spool = ctx.enter_context(tc.tile_pool(name="s", bufs=2))
        epool = ctx.enter_context(tc.tile_pool(name="e", bufs=3))
        atpool = ctx.enter_context(tc.tile_pool(name="at", bufs=8))
        opool_sb = ctx.enter_context(tc.tile_pool(name="osb", bufs=2))
        ps_mm = ctx.enter_context(tc.tile_pool(name="psmm", bufs=3, space="PSUM"))
        ps_pv = ctx.enter_context(tc.tile_pool(name="pspv", bufs=2, space="PSUM"))
        ps_sm = ctx.enter_context(tc.tile_pool(name="pssm", bufs=1, space="PSUM"))
        ps_o = ctx.enter_context(tc.tile_pool(name="pso", bufs=2, space="PSUM"))

        # ---- persistent tiles (loads emitted by the driver below) -------
        wq_g = [wpool.tile([128, 4 * QH * HD], MM, name=f"wqg{g}",
                           tag=f"wqg{g}") for g in range(4)]
        wkv_g = [wpool.tile([128, 8 * 2 * HD], MM, name=f"wkvg{g}",
                            tag=f"wkvg{g}") for g in range(2)]
        wo_t = wpool.tile([128, QH * H], MM, tag="wo")
        wq_ts = [wq_g[c // 4][:, (c % 4) * QH * HD:(c % 4 + 1) * QH * HD]
                 for c in range(NHC)]
        wkv_ts = [wkv_g[c // 8][:, (c % 8) * 2 * HD:(c % 8 + 1) * 2 * HD]
                  for c in range(NHC)]
        cos_t = wpool.tile([128, NSC * SC], F32, tag="cos")
        sin_t = wpool.tile([128, NSC * SC], MM, tag="sin")
        cs_loaded = [False] * NSC

        def cos_sl(sc):
            return cos_t[:, sc * SC:(sc + 1) * SC]

        def sin_sl(sc):
            return sin_t[:, sc * SC:(sc + 1) * SC]

        def load_cs(sc):
            if not cs_loaded[sc]:
                nc.sync.dma_start(cos_t[:, sc * SC:(sc + 1) * SC], cosT[sc])
                nc.sync.dma_start(sin_t[:, sc * SC:(sc + 1) * SC], sinT[sc])
                cs_loaded[sc] = True

        qsc_t = const.tile([1, QH + 1], F32, tag="qsc")
        qsb_t = const.tile([1, QH + 1], F32, tag="qsb")
        iwq_t = const.tile([128, 1], MM, tag="iwq")
        iwk_t = const.tile([128, 1], MM, tag="iwk")
        ones_t = const.tile([128, 1], MM, tag="ones")
        tri_t = const.tile([128, KT], MM, tag="tri", name="tri")
        khat = persist.tile([128, S], MM, tag="khat")
        v_sb = persist.tile([128, S], MM, tag="v")
        qhat = [persist.tile([128, S], MM, name=f"qhat{i}", tag=f"qhat{i}")
                for i in range(QH)]

        def load_wkv():
            for g in range(2):
                nc.sync.dma_start(wkv_g[g][:], wkvT[g])

        def load_wq_consts():
            for g in range(4):
                nc.sync.dma_start(wq_g[g][:], wqT[g])
            nc.sync.dma_start(qsc_t[:], qsc[:])
            nc.sync.dma_start(qsb_t[:], qsb[:])
            nc.sync.dma_start(iwq_t[:], iwq[:])
            nc.sync.dma_start(iwk_t[:], iwk[:])
            nc.sync.dma_start(tri_t[:], tri01[:])
            nc.vector.memset(ones_t[:], 1.0)

        def load_wo():
            for f in range(QH):
                nc.sync.dma_start(wo_t[:, f * H:(f + 1) * H],
                                  woT[f * HD:(f + 1) * HD, :])

        # norm+rope staged: s1 (right after the proj matmuls) does the
        # rotate-copy + cos-product (the last PSUM reads, so the proj bank
        # frees early) and the square on the scalar engine; the var matmul
        # (s2) trails by one projection group; s3 finishes rstd + rope.
        def norm_s1(pp, sc):
            sh = spool.tile([128, SC], MM, tag="sh", name="sh", bufs=4)
            nc.vector.tensor_copy(sh[0:64, :], pp[64:128, :])
            nc.vector.tensor_copy(sh[64:128, :], pp[0:64, :])
            sq = spool.tile([128, SC], MM, tag="sq", name="sq", bufs=4)
            nc.scalar.activation(sq[:], pp[:], AF.Square)
            uu = spool.tile([128, SC], MM, tag="uu", name="uu", bufs=4)
            nc.vector.tensor_mul(uu[:], pp[:], cos_sl(sc))
            return sh, sq, uu

        def norm_s2(sq, iw_t):
            var = ps_mm.tile([1, SC], F32, tag="mm", name="var")
            nc.tensor.matmul(var[:], iw_t[:], sq[:], start=True, stop=True)
            return var

        def norm_s3(sh, uu, var, sc, hd, hat_dst):
            # rstd = (var*qsc + qsb)^-0.5 via ln+exp (single act table set)
            hi = 0 if hd is None else hd + 1
            lv = spool.tile([1, SC], F32, tag="lv", name="lv")
            nc.scalar.activation(lv[:], var[:], AF.Ln,
                                 bias=qsb_t[:, hi:hi + 1],
                                 scale=qsc_t[:, hi:hi + 1])
            rs = spool.tile([1, SC], MM, tag="rs", name="rs")
            nc.scalar.activation(rs[:], lv[:], AF.Exp, scale=-0.5)
            # tt = rot(x) * sin' (sign pre-folded into sin'), s = uu + tt
            tt = spool.tile([128, SC], MM, tag="tt", name="tt")
            nc.vector.tensor_mul(tt[:], sh[:], sin_sl(sc))
            bb = spool.tile([128, SC], MM, tag="bb", name="bb")
            nc.gpsimd.partition_broadcast(bb[:], rs[:], 128)
            nc.vector.tensor_add(tt[:], tt[:], uu[:])
            nc.vector.tensor_mul(hat_dst, tt[:], bb[:])

        # ---- projections, per s-chunk -----------------------------------
        def hts_load(sc):
            tiles = []
            for g in range(4):
                t = hpool.tile([128, 4 * SC], MM, tag="ht", name="ht")
                nc.sync.dma_start(t[:], hT[sc, g])
                tiles.append(t)
            return [tiles[c // 4][:, (c % 4) * SC:(c % 4 + 1) * SC]
                    for c in range(NHC)]

        def proj_chunk(sc, hts, carry=None):
            # five projections (k, q0..q3), staged so each var matmul is
            # emitted after the NEXT projection's matmul group; v-proj is
            # emitted between k and q0 so it only depends on wkv + hT.
            # The last two finishers are RETURNED (not emitted) so the
            # caller can interleave their M=1 var matmuls into the next
            # phase's PE stream instead of head-of-line blocking on the
            # scalar engine's Square at the chunk boundary.
            specs = [(iwk_t, None, khat)] + [
                (iwq_t, hd, qhat[hd]) for hd in range(QH)]
            state = []  # (sh, sq, uu, spec)

            def do_mm(idx):
                pp = ps_mm.tile([128, SC], F32, tag="mm", name="pp")
                for c in range(NHC):
                    if idx == 0:
                        w_sl = wkv_ts[c][:, 0:HD]
                    else:
                        w_sl = wq_ts[c][:, (idx - 1) * HD:idx * HD]
                    nc.tensor.matmul(pp[:], w_sl, hts[c][:],
                                     start=(c == 0), stop=(c == NHC - 1))
                sh, sq, uu = norm_s1(pp, sc)
                state.append((sh, sq, uu, specs[idx]))

            def finish_one():
                sh, sq, uu, (iw_t, hd, dst) = state.pop(0)
                var = norm_s2(sq, iw_t)
                norm_s3(sh, uu, var, sc, hd, dst[:, sc * SC:(sc + 1) * SC])

            def v_proj():
                for ss in range(4):
                    vp = ps_o.tile([128, SC], F32, tag="o", name="vp")
                    for c in range(NHC):
                        nc.tensor.matmul(vp[:, 0:HD],
                                         hts[c][:, ss * 128:(ss + 1) * 128],
                                         wkv_ts[c][:, HD:2 * HD],
                                         start=(c == 0), stop=(c == NHC - 1))
                    col = (sc * 4 + ss) * 128
                    nc.vector.tensor_copy(v_sb[:, col:col + 128], vp[:, 0:HD])

            do_mm(0)
            if carry:
                for fin in carry:
                    fin()
            v_proj()
            for idx in range(1, 5):
                do_mm(idx)
                if idx <= 3:
                    finish_one()
            return [finish_one, finish_one]

        # ---- attention + o_proj, per q-tile ------------------------------
        mask_starts = [sum(mask_counts[:i]) for i in range(NSC)]

        def attn_qtile(qi, carry=None):
            ats = []
            mask_idx = mask_starts[qi]
            kts = [kt for kt in range(NKT) if plan[qi][kt] != "skip"]
            mtiles = {}
            for kt in kts:
                if plan[qi][kt] == "mask":
                    mt = mpool.tile([128, SC], F32, tag="mask", name="mk")
                    nc.sync.dma_start(mt[:], mblk[mask_idx])
                    mtiles[kt] = mt
                    mask_idx += 1
            for hd in range(QH):
                qsl = qhat[hd][:, qi * SC:(qi + 1) * SC]
                pv = ps_pv.tile([128, SC], F32, tag="pv")
                es = ps_sm.tile([1, SC], F32, tag="es")
                sts = {}
                pend = []
                esn = [0, 0]  # groups emitted, total groups
                ngroups = (len(kts) + 3) // 4
                esn[1] = ngroups

                def tail(j):
                    kt = kts[j]
                    st, c0 = sts.pop(j)
                    ex = epool.tile([128, SC], MM, tag="ex", name="ex",
                                    bufs=5)
                    nc.scalar.activation(ex[:, c0:SC], st[:, c0:SC], AF.Exp)
                    if c0:
                        nc.vector.memset(ex[:, 0:c0], 0.0)
                        nc.vector.tensor_mul(ex[:, c0:c0 + KT],
                                             ex[:, c0:c0 + KT], tri_t[:])
                    elif plan[qi][kt] == "diag0":
                        nc.vector.tensor_mul(ex[:, 0:KT], ex[:, 0:KT],
                                             tri_t[:])
                    last = j == len(kts) - 1
                    nc.tensor.matmul(pv[:, c0:SC],
                                     v_sb[:, kt * 128:(kt + 1) * 128],
                                     ex[:, c0:SC], start=(j == 0), stop=last)
                    pend.append(ex)
                    if len(pend) == 4 or last:
                        if len(pend) == 1:
                            red = pend[0]
                        else:
                            red = epool.tile([128, SC], MM, tag="exs",
                                             name="exs", bufs=2)
                            nc.vector.tensor_add(red[:], pend[0][:],
                                                 pend[1][:])
                            if len(pend) == 4:
                                red2 = epool.tile([128, SC], MM, tag="exs2",
                                                  name="exs2", bufs=2)
                                nc.vector.tensor_add(red2[:], pend[2][:],
                                                     pend[3][:])
                                nc.vector.tensor_add(red[:], red[:], red2[:])
                            elif len(pend) == 3:
                                nc.vector.tensor_add(red[:], red[:],
                                                     pend[2][:])
                        nc.tensor.matmul(es[:], ones_t[:], red[:],
                                         start=(esn[0] == 0),
                                         stop=(esn[0] == esn[1] - 1))
                        esn[0] += 1
                        pend.clear()

                # pipeline QK^T one k-tile ahead of exp/PV
                for j, kt in enumerate(kts):
                    kind = plan[qi][kt]
                    c0 = int(kind[4]) * KT if kind.startswith("diag") else 0
                    st = ps_mm.tile([128, SC], F32, tag="mm")
                    nc.tensor.matmul(st[:, c0:SC],
                                     khat[:, kt * 128:(kt + 1) * 128],
                                     qsl[:, c0:SC], start=True, stop=True)
                    if kind == "mask":
                        nc.vector.tensor_add(st[:], st[:], mtiles[kt][:])
                    sts[j] = (st, c0)
                    if j >= 1:
                        tail(j - 1)
                tail(len(kts) - 1)
                rs = spool.tile([1, SC], F32, tag="ars")
                nc.vector.reciprocal_approx_fast(rs[:], es[:])
                bb = spool.tile([128, SC], F32, tag="abb")
                nc.gpsimd.partition_broadcast(bb[:], rs[:], 128)
                at = atpool.tile([128, SC], MM, tag="at")
                nc.vector.tensor_mul(at[:], pv[:], bb[:])
                ats.append(at)
                if hd == 0 and carry:
                    for fin in carry:
                        fin()
                    carry = None
            # o_proj for this q-tile
            for ss in range(4):
                ob = opool_sb.tile([128, H], MM, tag="osb", name="ob")
                for ho in range(4):
                    op_t = ps_o.tile([128, SC], F32, tag="o", name="op")
                    for hd in range(QH):
                        nc.tensor.matmul(
                            op_t[:],
                            ats[hd][:, ss * 128:(ss + 1) * 128],
                            wo_t[:, hd * H + ho * SC:hd * H + (ho + 1) * SC],
                            start=(hd == 0), stop=(hd == QH - 1))
                    if ho % 2 == 0:
                        nc.scalar.copy(ob[:, ho * SC:(ho + 1) * SC], op_t[:])
                    else:
                        nc.vector.tensor_copy(ob[:, ho * SC:(ho + 1) * SC],
                                              op_t[:])
                    if qi == NSC - 1:
                        # fire each 512-col slice as soon as its copy lands
                        # so the tail DMA isn't serialized behind all 4
                        nc.gpsimd.dma_start(
                            out[qi * SC + ss * 128:qi * SC + (ss + 1) * 128,
                                ho * SC:(ho + 1) * SC],
                            ob[:, ho * SC:(ho + 1) * SC])
                if qi != NSC - 1:
                    nc.gpsimd.dma_start(
                        out[qi * SC + ss * 128:qi * SC + (ss + 1) * 128, :],
                        ob[:])

        # ---- driver: software-pipelined phase order ----------------------
        load_wkv()
        hts0 = hts_load(0)
        load_cs(0)
        load_wq_consts()
        p0 = proj_chunk(0, hts0)
        hts1 = hts_load(1)
        load_cs(1)
        p1 = proj_chunk(1, hts1, carry=p0)
        load_wo()
        attn_qtile(0, carry=p1)
        hts2 = hts_load(2)
        load_cs(2)
        p2 = proj_chunk(2, hts2)
        attn_qtile(1, carry=p2)
        hts3 = hts_load(3)
        load_cs(3)
        p3 = proj_chunk(3, hts3)
        attn_qtile(2, carry=p3)
        attn_qtile(3)


def _causal_diag_j(blk, qi, kt):
    """Return j in 0..3 if the block matches the canonical causal step at
    diagonal offset (kt == 4*qi + j), else None. blk: [B, SC, KT]."""
    j = kt - 4 * qi
    if not (0 <= j <= 3):
        return None
    q_idx = qi * SC + np.arange(SC)[:, None]
    k_idx = kt * KT + np.arange(KT)[None, :]
    want = np.where(k_idx > q_idx, np.float32(-1e9), np.float32(0.0))
    return j if bool((blk == want[None]).all()) else None


def _mask_plan(mask):
    """Classify [qi][kt] blocks of the (q,k) mask, unified across batch."""
    plan = []
    for qi in range(NSC):
        row = []
        for kt in range(NKT):
            blk = mask[:, 0, qi * SC:(qi + 1) * SC, kt * KT:(kt + 1) * KT]
            if (blk <= SKIP_THRESH).all():
                row.append("skip")
            elif (blk == 0.0).all():
                row.append("zero")
            else:
                j = _causal_diag_j(blk, qi, kt)
                row.append(f"diag{j}" if j is not None else "mask")
        # guard: a q-tile with no included block would divide by zero
        if all(s == "skip" for s in row):
            row[0] = "mask"
        plan.append(row)
    return plan


def kernel(hidden_states, cos, sin, attention_mask, wq, wk, wv, wo,
           q_norm_w, k_norm_w, ssmax_scale):
    global LAST_EXEC_NS
    import os
    import ml_dtypes
    from concourse.bass_utils import run_bass_kernel_spmd

    f32 = np.float32
    hidden_states = np.asarray(hidden_states, f32)
    cos = np.asarray(cos, f32)
    sin = np.asarray(sin, f32)
    attention_mask = np.asarray(attention_mask, f32)
    wq = np.asarray(wq, f32)
    wk = np.asarray(wk, f32)
    wv = np.asarray(wv, f32)
    wo = np.asarray(wo, f32)
    q_norm_w = np.asarray(q_norm_w, f32)
    k_norm_w = np.asarray(k_norm_w, f32)
    ssmax = np.asarray(ssmax_scale, f32).reshape(NH)

    plan = _mask_plan(attention_mask)
    mask_counts = [sum(1 for s in row if s == "mask") for row in plan]
    key = (tuple(tuple(r) for r in plan),)
    if key not in _compiled_cache:
        _compiled_cache[key] = _build_program(plan, mask_counts)
    nc = _compiled_cache[key]

    bf16 = ml_dtypes.bfloat16
    qw = np.tile(q_norm_w, QH)
    iwq_np = (1.0 / (HD * q_norm_w ** 2)).astype(bf16)[:, None]
    iwk_np = (1.0 / (HD * k_norm_w ** 2)).astype(bf16)[:, None]
    # cos kept f32; sin gets rotate_half's sign fold: sin'[d<64] = -sin[d]
    sinp = sin.T.copy()                       # [HD, S]
    sinp[:64] = -sinp[:64]
    cos_np = np.ascontiguousarray(
        cos.T.reshape(HD, NSC, SC).transpose(1, 0, 2))        # [NSC, HD, SC]
    sin_np = np.ascontiguousarray(
        sinp.reshape(HD, NSC, SC).transpose(1, 0, 2)).astype(bf16)
    tri_np = (np.arange(KT)[:, None] <= np.arange(KT)[None, :]) \
        .astype(bf16)                                          # keep r <= c

    in_maps = []
    for core in range(NCORES):
        b, g = divmod(core, TP)
        hTm = np.ascontiguousarray(
            hidden_states[b].T.reshape(4, 4, HC, NSC, SC)
            .transpose(3, 0, 2, 1, 4).reshape(NSC, 4, HC, 4 * SC)
        ).astype(bf16)
        wq_s = wq[g * QH * HD:(g + 1) * QH * HD] * qw[:, None]
        wk_s = wk[g * HD:(g + 1) * HD] * k_norm_w[:, None]
        wv_s = wv[g * HD:(g + 1) * HD]
        wo_s = wo[:, g * QH * HD:(g + 1) * QH * HD]
        qcv = np.array([ssmax[g * QH + i] * math.log(S) / math.sqrt(HD)
                        for i in range(QH)], f32)
        # entry 0 is the k-norm (qc=1); entries 1..QH are the q heads
        qcall = np.concatenate([[1.0], qcv]).astype(f32)
        qsc_np = (1.0 / qcall ** 2)[None, :].astype(f32)
        qsb_np = (EPS / qcall ** 2)[None, :].astype(f32)
        wqTm = np.ascontiguousarray(
            wq_s.T.reshape(4, 4, HC, QH * HD)
            .transpose(0, 2, 1, 3).reshape(4, HC, 4 * QH * HD)).astype(bf16)
        wkv = np.concatenate(
            [wk_s.T.reshape(NHC, HC, 1, HD), wv_s.T.reshape(NHC, HC, 1, HD)],
            axis=2)  # [NHC, HC, 2, HD]
        wkvm = np.ascontiguousarray(
            wkv.reshape(2, 8, HC, 2 * HD).transpose(0, 2, 1, 3)
            .reshape(2, HC, 8 * 2 * HD)).astype(bf16)
        m = {
            "hT": hTm,
            "wqT": wqTm,
            "wkvT": wkvm,
            "woT": np.ascontiguousarray(wo_s.T).astype(bf16),
            "cosT": cos_np, "sinT": sin_np,
            "qsc": qsc_np, "qsb": qsb_np, "iwq": iwq_np, "iwk": iwk_np,
            "tri01": tri_np,
        }
        n_mask = sum(mask_counts)
        if n_mask:
            blocks = np.zeros((n_mask, KT, SC), f32)
            i = 0
            for qi in range(NSC):
                for kt in range(NKT):
                    if plan[qi][kt] != "mask":
                        continue
                    blocks[i] = attention_mask[
                        b, 0, qi * SC:(qi + 1) * SC,
                        kt * KT:(kt + 1) * KT].T
                    i += 1
            m["mblk"] = blocks
        in_maps.append(m)

    trace = bool(int(os.environ.get("BASS_KERNEL_TRACE", "0")))
    res = run_bass_kernel_spmd(nc, in_maps, list(range(NCORES)), trace=trace)
    LAST_EXEC_NS = res.exec_time_ns
    globals()["LAST_RESULTS"] = res

    final = np.zeros((B, S, H), f32)
    for core in range(NCORES):
        b = core // TP
        final[b] += np.asarray(res.results[core]["out"], f32)
    return final
